# revision 16
# baseline (speedup 1.0000x reference)
"""Trainium2 Bass kernel: MultiHeadSpatioTemporalAttention (16 heads, d=1024)
+ residual + LayerNorm, returning (y, attn_probs).

Sharding: tensor-parallel over heads — core c owns heads {2c, 2c+1} for both
batch elements. Q/K/V projections are column-parallel (each core computes its
128 channels for all 4096 tokens); attention + softmax + attn@V are fully
local per head; the output projection is handled by AllGather-ing the
per-head context (channel-sharded) and having each core compute the Wo
projection + bias + residual + LayerNorm for its own 512-token row shard.

Matmuls run as float32r (TF32-like full-rate mode). Softmax row sums come for
free from the Exp activation's accum_out; normalization of the attention
probabilities is a per-partition DVE multiply. The attn@V contraction uses a
second, transposed score computation (S^T tiles) so the contraction dim lands
on partitions, avoiding any on-chip transposes.
"""

import os
import sys

if "jax" not in sys.modules:
    # The bass SPMD runner executes through the axon PJRT proxy; make sure a
    # harness-level JAX_PLATFORMS=cpu doesn't hide the NeuronCores.
    if os.environ.get("AXON_H4_ENABLED") == "1":
        os.environ["JAX_PLATFORMS"] = "axon"

import numpy as np

import concourse.bass as bass
import concourse.mybir as mybir
import concourse.tile as tile
from concourse import bacc
from concourse.bass_utils import run_bass_kernel_spmd

F32 = mybir.dt.float32
F32R = mybir.dt.float32r
EXP = mybir.ActivationFunctionType.Exp
SQRT = mybir.ActivationFunctionType.Sqrt
ALU = mybir.AluOpType

N_CORES = 8
B, S, D, H, DK = 2, 2048, 1024, 16, 64
T = B * S                # 4096 tokens total
TSH = T // N_CORES       # 512 tokens per core (output row shard)
P = 128
DC = D // P              # 8 d-chunks
QT = S // P              # 16 q-tiles per batch
KB = S // 512            # 4 k-blocks per batch
NTT = T // P             # 32 token tiles
LN_EPS = 1e-5
SCALE = 1.0 / np.sqrt(DK)  # 0.125


def r(ap):
    return ap.bitcast(F32R)


def build_kernel():
    nc = bacc.Bacc("TRN2", target_bir_lowering=False, debug=False,
                   enable_asserts=False, num_devices=N_CORES)

    xT = nc.dram_tensor("xT", [D, T], F32, kind="ExternalInput").ap()
    xrows = nc.dram_tensor("xrows", [TSH, D], F32, kind="ExternalInput").ap()
    wqT = nc.dram_tensor("wqT", [D, P], F32, kind="ExternalInput").ap()
    wkT = nc.dram_tensor("wkT", [D, P], F32, kind="ExternalInput").ap()
    wvT = nc.dram_tensor("wvT", [D, P], F32, kind="ExternalInput").ap()
    woT = nc.dram_tensor("woT", [D, D], F32, kind="ExternalInput").ap()
    bo = nc.dram_tensor("bo", [1, D], F32, kind="ExternalInput").ap()
    gamma = nc.dram_tensor("gamma", [1, D], F32, kind="ExternalInput").ap()
    beta = nc.dram_tensor("beta", [1, D], F32, kind="ExternalInput").ap()
    ident_in = nc.dram_tensor("ident_in", [P, P], F32, kind="ExternalInput").ap()
    ones_in = nc.dram_tensor("ones_in", [1, P], F32, kind="ExternalInput").ap()

    attn_o = nc.dram_tensor("attn_o", [B, 2, S, S], F32, kind="ExternalOutput").ap()
    y_o = nc.dram_tensor("y_o", [TSH, D], F32, kind="ExternalOutput").ap()

    with tile.TileContext(nc) as tc:
        with tc.tile_pool(name="big", bufs=1) as big, \
             tc.tile_pool(name="wts", bufs=1) as wts, \
             tc.tile_pool(name="xtp", bufs=8) as xtp, \
             tc.tile_pool(name="ep", bufs=3) as ep, \
             tc.tile_pool(name="etp", bufs=4) as etp, \
             tc.tile_pool(name="sm", bufs=8) as sm, \
             tc.tile_pool(name="ps", bufs=1, space="PSUM") as ps, \
             tc.tile_pool(name="dram", bufs=1, space="DRAM") as dram:

            # ---- persistent SBUF ----
            qt_sb = big.tile([P, T], F32R)            # Q^T  [chan, tok]
            kt_sb = big.tile([P, T], F32R)            # K^T  [chan, tok]
            v_sb = big.tile([P, NTT, P], F32R)        # V    [tok%128, tok//128, chan]
            wo_sb = big.tile([P, DC, D], F32R)        # Wo^T [c%128, c//128, d]
            recips_sb = big.tile([P, 4 * QT], F32)   # 1/rowsum, col=(bh*16+qt)
            gam_bc = big.tile([P, D], F32)
            bet_bc = big.tile([P, D], F32)

            wq_sb = wts.tile([P, DC, P], F32R)
            wk_sb = wts.tile([P, DC, P], F32R)
            wv_sb = wts.tile([P, DC, P], F32R)
            ones_sb = wts.tile([1, P], F32R)
            ident = wts.tile([P, P], F32R)
            bo_sb = wts.tile([1, D], F32R)
            gam_sb = wts.tile([1, D], F32R)
            bet_sb = wts.tile([1, D], F32R)

            nc.sync.dma_start(wq_sb[:], wqT.rearrange("(dc p) c -> p dc c", p=P).bitcast(F32R))
            nc.sync.dma_start(wk_sb[:], wkT.rearrange("(dc p) c -> p dc c", p=P).bitcast(F32R))
            nc.sync.dma_start(wv_sb[:], wvT.rearrange("(dc p) c -> p dc c", p=P).bitcast(F32R))
            nc.sync.dma_start(wo_sb[:], woT.rearrange("(dc p) d -> p dc d", p=P).bitcast(F32R))
            nc.sync.dma_start(bo_sb[:], bo.bitcast(F32R))
            nc.sync.dma_start(gam_sb[:], gamma.bitcast(F32R))
            nc.sync.dma_start(bet_sb[:], beta.bitcast(F32R))
            eps_col = wts.tile([P, 1], F32)
            nc.sync.dma_start(ones_sb[:], ones_in.bitcast(F32R))
            nc.gpsimd.memset(eps_col[:], LN_EPS)
            nc.sync.dma_start(ident[:], ident_in.bitcast(F32R))

            # ---- Stage A: Q^T, K^T (chan-major) and V (token-major) ----
            for tb in range(8):           # token blocks of 512
                xts = []
                for dc in range(DC):
                    xt = xtp.tile([P, 512], F32R, name=f"xt_{tb}_{dc}", tag="xt")
                    nc.sync.dma_start(xt[:], xT[dc * P:(dc + 1) * P,
                                                tb * 512:(tb + 1) * 512].bitcast(F32R))
                    xts.append(xt)
                qp = ps.tile([P, 512], F32, tag="ps512a", name=f"qp{tb}", bufs=2)
                kp = ps.tile([P, 512], F32, tag="ps512a", name=f"kp{tb}", bufs=2)
                for dc in range(DC):
                    nc.tensor.matmul(qp[:], lhsT=(wq_sb[:, dc, :]), rhs=(xts[dc][:]),
                                     start=(dc == 0), stop=(dc == DC - 1))
                for dc in range(DC):
                    nc.tensor.matmul(kp[:], lhsT=(wk_sb[:, dc, :]), rhs=(xts[dc][:]),
                                     start=(dc == 0), stop=(dc == DC - 1))
                nc.scalar.copy(qt_sb[:, tb * 512:(tb + 1) * 512], qp[:])
                nc.scalar.copy(kt_sb[:, tb * 512:(tb + 1) * 512], kp[:])
                for i in range(4):        # token tiles of 128 within the block
                    vp = ps.tile([P, 512], F32, tag="ps512b", name=f"vp{tb}_{i}",
                                 bufs=2)
                    for dc in range(DC):
                        nc.tensor.matmul(vp[:, 0:P],
                                         lhsT=(xts[dc][:, i * P:(i + 1) * P]),
                                         rhs=(wv_sb[:, dc, :]),
                                         start=(dc == 0), stop=(dc == DC - 1))
                    nc.vector.tensor_copy(v_sb[:, tb * 4 + i, :], vp[:, 0:P])

            # ---- Stage C: scores (q-major), fused softmax, attn output ----
            for b in range(2):
                for qt in range(QT):
                    for h in range(2):
                        ch0 = h * DK
                        bh = b * 2 + h
                        sp = ps.tile([P, S], F32, tag="psbig", name=f"sp{bh}_{qt}")
                        q_lhsT = qt_sb[ch0:ch0 + DK,
                                       b * S + qt * P: b * S + (qt + 1) * P]
                        for kb in range(KB):
                            nc.tensor.matmul(
                                sp[:, kb * 512:(kb + 1) * 512],
                                lhsT=(q_lhsT),
                                rhs=(kt_sb[ch0:ch0 + DK,
                                            b * S + kb * 512: b * S + (kb + 1) * 512]),
                                start=True, stop=True)
                        e_sb = ep.tile([P, S], F32, name=f"e{bh}_{qt}", tag="e")
                        sums = sm.tile([P, 1], F32, name=f"sums{bh}_{qt}", tag="sums")
                        nc.scalar.activation(e_sb[:], sp[:], EXP, scale=SCALE,
                                             accum_out=sums[:])
                        rcol = recips_sb[:, bh * QT + qt: bh * QT + qt + 1]
                        nc.vector.reciprocal(rcol, sums[:])
                        nc.vector.tensor_scalar_mul(e_sb[:], e_sb[:], rcol)
                        nc.sync.dma_start(
                            attn_o[b, h, qt * P:(qt + 1) * P, :], e_sb[:])

            # ---- flip recips to row layout via DRAM round-trip ----
            recipD = dram.tile([4 * QT * P], F32)
            nc.sync.dma_start(recipD.rearrange("(c p) -> p c", p=P), recips_sb[:])

            a2a_in = dram.tile([N_CORES, P, TSH], F32)
            a2a_out = dram.tile([N_CORES, P, TSH], F32)

            # ---- Stage B: S^T, exp, attn@V (ctx^T), normalize ----
            for b in range(2):
                for qb in range(KB):      # q blocks of 512
                    cxps = [ps.tile([DK, 512], F32, tag="ps512b",
                                    name=f"cx{b}_{qb}_{h}", bufs=2)
                            for h in range(2)]
                    for kt in range(QT):  # k tiles of 128
                        for h in range(2):
                            ch0 = h * DK
                            stp = ps.tile([P, 512], F32, tag="ps512a",
                                          name=f"st{b}_{qb}_{kt}_{h}", bufs=2)
                            nc.tensor.matmul(
                                stp[:],
                                lhsT=(kt_sb[ch0:ch0 + DK,
                                             b * S + kt * P: b * S + (kt + 1) * P]),
                                rhs=(qt_sb[ch0:ch0 + DK,
                                            b * S + qb * 512: b * S + (qb + 1) * 512]),
                                start=True, stop=True)
                            et = etp.tile([P, 512], F32R,
                                          name=f"et{b}_{qb}_{kt}_{h}", tag="et")
                            nc.scalar.activation(et[:], stp[:], EXP, scale=SCALE)
                            nc.tensor.matmul(
                                cxps[h][:],
                                lhsT=(v_sb[:, b * QT + kt, ch0:ch0 + DK]),
                                rhs=(et[:]),
                                start=(kt == 0), stop=(kt == QT - 1))
                    for h in range(2):
                        bh = b * 2 + h
                        rr = sm.tile([1, 512], F32R, name=f"rr{b}_{qb}_{h}",
                                     tag="rr", bufs=2)
                        nc.sync.dma_start(
                            rr[:], recipD[None, bh * S + qb * 512:
                                          bh * S + (qb + 1) * 512].bitcast(F32R))
                        bc = ps.tile([DK, 512], F32, tag="ps512a",
                                     name=f"bc{b}_{qb}_{h}", bufs=2)
                        nc.tensor.matmul(bc[:], lhsT=(ones_sb[0:1, 0:DK]),
                                         rhs=(rr[:]), start=True, stop=True)
                        bc_sb = etp.tile([DK, 512], F32, name=f"bcs{b}_{qb}_{h}",
                                         tag="bcs", bufs=2)
                        nc.scalar.copy(bc_sb[:], bc[:])
                        cst = etp.tile([DK, 512], F32, name=f"cst{b}_{qb}_{h}",
                                       tag="cst", bufs=2)
                        nc.vector.tensor_mul(cst[:], cxps[h][:], bc_sb[:])
                        nc.sync.dma_start(
                            a2a_in[b * KB + qb, h * DK:(h + 1) * DK, :], cst[:])

            # ---- AllToAll: channel-sharded ctx^T -> token-sharded full ctx^T ----
            # in[j] = my 128 channels for token block j; out[rk] = rank rk's
            # 128 channels (= global channels 128*rk..) for MY token shard.
            nc.gpsimd.collective_compute(
                "AllToAll", ALU.bypass,
                replica_groups=[list(range(N_CORES))],
                ins=[a2a_in.opt()], outs=[a2a_out.opt()])

            # gamma/beta broadcast to all partitions (PE ones-matmul)
            gbp = ps.tile([P, S], F32, tag="psbig", name="gbp")
            for half in range(2):
                nc.tensor.matmul(gbp[:, half * 512:(half + 1) * 512],
                                 lhsT=(ones_sb[0:1, :]),
                                 rhs=(gam_sb[0:1, half * 512:(half + 1) * 512]),
                                 start=True, stop=True)
                nc.tensor.matmul(gbp[:, D + half * 512: D + (half + 1) * 512],
                                 lhsT=(ones_sb[0:1, :]),
                                 rhs=(bet_sb[0:1, half * 512:(half + 1) * 512]),
                                 start=True, stop=True)
            nc.vector.tensor_copy(gam_bc[:], gbp[:, 0:D])
            nc.vector.tensor_copy(bet_bc[:], gbp[:, D:2 * D])

            # ---- Stage D/E: Wo projection + bias + residual + LayerNorm ----
            for i in range(4):       # token tiles of 128 in the shard
                op = ps.tile([P, S], F32, tag="psbig", name=f"op{i}")
                ctx_ts = []
                for rk in range(N_CORES):
                    ct = sm.tile([P, P], F32R, name=f"ct{i}_{rk}", tag="ct", bufs=16)
                    nc.sync.dma_start(ct[:], a2a_out[rk, :, i * P:(i + 1) * P].bitcast(F32R))
                    ctx_ts.append(ct)
                xr_t = etp.tile([P, D], F32R, name=f"xr{i}", tag="xr", bufs=2)
                nc.sync.dma_start(xr_t[:], xrows[i * P:(i + 1) * P, :].bitcast(F32R))
                for half in range(2):
                    hs = slice(half * 512, (half + 1) * 512)
                    for rk in range(N_CORES):
                        nc.tensor.matmul(op[:, hs], lhsT=(ctx_ts[rk][:]),
                                         rhs=(wo_sb[:, rk, hs]),
                                         start=(rk == 0), stop=False)
                    nc.tensor.matmul(op[:, hs], lhsT=(ones_sb[0:1, :]),
                                     rhs=(bo_sb[0:1, hs]),
                                     start=False, stop=False)
                    nc.tensor.matmul(op[:, hs], lhsT=(ident[:]),
                                     rhs=(xr_t[:, hs]),
                                     start=False, stop=True)
                # LayerNorm over free dim (D=1024)
                musum = sm.tile([P, 1], F32, name=f"musum{i}", tag="mu")
                nc.vector.reduce_sum(musum[:], op[:, 0:D], axis=mybir.AxisListType.X)
                mu = sm.tile([P, 1], F32, name=f"mu{i}", tag="mu")
                nc.vector.tensor_scalar_mul(mu[:], musum[:], 1.0 / D)
                oc = ep.tile([P, S], F32, name=f"oc{i}", tag="e")
                nc.vector.tensor_scalar(oc[:, 0:D], op[:, 0:D], mu[:], None,
                                        op0=ALU.subtract)
                sqsum = sm.tile([P, 1], F32, name=f"sqsum{i}", tag="mu")
                nc.vector.scalar_tensor_tensor(
                    oc[:, D:2 * D], in0=oc[:, 0:D], scalar=0.0, in1=oc[:, 0:D],
                    op0=ALU.add, op1=ALU.mult, accum_out=sqsum[:])
                sd = sm.tile([P, 1], F32, name=f"sd{i}", tag="mu")
                nc.scalar.activation(sd[:], sqsum[:], SQRT,
                                     scale=1.0 / D, bias=eps_col[:])
                rstd = sm.tile([P, 1], F32, name=f"rstd{i}", tag="mu")
                nc.vector.reciprocal(rstd[:], sd[:])
                y_sb = etp.tile([P, D], F32, name=f"y{i}", tag="y", bufs=2)
                nc.vector.scalar_tensor_tensor(
                    y_sb[:], in0=oc[:, 0:D], scalar=rstd[:], in1=gam_bc[:],
                    op0=ALU.mult, op1=ALU.mult)
                nc.vector.tensor_add(y_sb[:], y_sb[:], bet_bc[:])
                nc.sync.dma_start(y_o[i * P:(i + 1) * P, :], y_sb[:])

    nc.compile()
    return nc


_NC_CACHE = {}


def _get_nc():
    if "nc" not in _NC_CACHE:
        _NC_CACHE["nc"] = build_kernel()
    return _NC_CACHE["nc"]


def make_in_maps(x, Wq, Wk, Wv, Wo, bo, gamma, beta):
    x = np.asarray(x, dtype=np.float32)
    xf = np.ascontiguousarray(x.reshape(T, D))
    xT = np.ascontiguousarray(xf.T)
    woT = np.ascontiguousarray(np.asarray(Wo, np.float32).T)
    bo2 = np.ascontiguousarray(np.asarray(bo, np.float32).reshape(1, D))
    g2 = np.ascontiguousarray(np.asarray(gamma, np.float32).reshape(1, D))
    b2 = np.ascontiguousarray(np.asarray(beta, np.float32).reshape(1, D))
    in_maps = []
    for c in range(N_CORES):
        rs = slice(2 * c * DK, 2 * (c + 1) * DK)
        in_maps.append({
            "xT": xT,
            "xrows": np.ascontiguousarray(xf[c * TSH:(c + 1) * TSH]),
            "wqT": np.ascontiguousarray(np.asarray(Wq, np.float32)[rs].T),
            "wkT": np.ascontiguousarray(np.asarray(Wk, np.float32)[rs].T),
            "wvT": np.ascontiguousarray(np.asarray(Wv, np.float32)[rs].T),
            "woT": woT,
            "bo": bo2,
            "gamma": g2,
            "beta": b2,
            "ident_in": np.eye(P, dtype=np.float32),
            "ones_in": np.ones((1, P), dtype=np.float32),
        })
    return in_maps


def assemble_outputs(results):
    y = np.concatenate([results[c]["y_o"] for c in range(N_CORES)], axis=0)
    y = y.reshape(B, S, D)
    attn = np.stack([results[c]["attn_o"] for c in range(N_CORES)], axis=0)
    # [8, B, 2, S, S] -> [B, 16, S, S]
    attn = attn.transpose(1, 0, 2, 3, 4).reshape(B, H, S, S)
    return y, attn


def kernel(x, Wq, Wk, Wv, Wo, bo, gamma, beta):
    nc = _get_nc()
    in_maps = make_in_maps(x, Wq, Wk, Wv, Wo, bo, gamma, beta)
    res = run_bass_kernel_spmd(nc, in_maps, core_ids=list(range(N_CORES)))
    return assemble_outputs(res.results)


# revision 20
# speedup vs baseline: 1.3966x; 1.3966x over previous
"""Trainium2 Bass kernel: MultiHeadSpatioTemporalAttention (16 heads, d=1024)
+ residual + LayerNorm, returning (y, attn_probs).

Sharding: tensor-parallel over heads — core c owns heads {2c, 2c+1} for both
batch elements. Q/K/V projections are column-parallel (each core computes its
128 channels for all 4096 tokens); attention + softmax + attn@V are fully
local per head; the output projection runs after an AllToAll that turns the
channel-sharded context into a token-sharded full context, so each core does
Wo + bias + residual + LayerNorm for its own 512-token row shard.

Attention is computed once, in k-major (transposed) orientation: S^T tiles
[k,q] come straight off the PE, exp runs on ScalarE, the attn@V contraction
consumes the exp tiles directly (contraction dim on partitions, no on-chip
transposes), and a ones-column prepended to V yields the softmax row sums in
the same matmul. Normalization is one DVE multiply against a PE-broadcast
reciprocal row. The attention output is written in [k,q] layout and the host
reassembles the [q,k] tensor with a numpy transpose.

All matmuls run as float32r (TF32-like full-rate PE mode); accumulation is
fp32 in PSUM.
"""

import os
import sys

if "jax" not in sys.modules:
    # The bass SPMD runner executes through the axon PJRT proxy; make sure a
    # harness-level JAX_PLATFORMS=cpu doesn't hide the NeuronCores.
    if os.environ.get("AXON_H4_ENABLED") == "1":
        os.environ["JAX_PLATFORMS"] = "axon"

import numpy as np

import concourse.bass as bass
import concourse.mybir as mybir
import concourse.tile as tile
from concourse import bacc
from concourse.bass_utils import run_bass_kernel_spmd

F32 = mybir.dt.float32
F32R = mybir.dt.float32r
U32 = mybir.dt.uint32
EXP = mybir.ActivationFunctionType.Exp
SQRT = mybir.ActivationFunctionType.Sqrt
ALU = mybir.AluOpType

N_CORES = 8
B, S, D, H, DK = 2, 2048, 1024, 16, 64
T = B * S                # 4096 tokens total
TSH = T // N_CORES       # 512 tokens per core (output row shard)
P = 128
DC = D // P              # 8 d-chunks
QT = S // P              # 16 tiles of 128 per batch
KB = S // 512            # 4 blocks of 512 per batch
NTT = T // P             # 32 token tiles
LN_EPS = 1e-5
SCALE = 1.0 / np.sqrt(DK)  # 0.125
ONE_U32 = 0x3F800000     # 1.0f bit pattern


def build_kernel():
    nc = bacc.Bacc("TRN2", target_bir_lowering=False, debug=False,
                   enable_asserts=False, num_devices=N_CORES)

    xT = nc.dram_tensor("xT", [D, T], F32, kind="ExternalInput").ap()
    xrows = nc.dram_tensor("xrows", [TSH, D], F32, kind="ExternalInput").ap()
    wqT = nc.dram_tensor("wqT", [D, P], F32, kind="ExternalInput").ap()
    wkT = nc.dram_tensor("wkT", [D, P], F32, kind="ExternalInput").ap()
    wvT = nc.dram_tensor("wvT", [D, P], F32, kind="ExternalInput").ap()
    woT = nc.dram_tensor("woT", [D, D], F32, kind="ExternalInput").ap()
    bo = nc.dram_tensor("bo", [1, D], F32, kind="ExternalInput").ap()
    gamma = nc.dram_tensor("gamma", [1, D], F32, kind="ExternalInput").ap()
    beta = nc.dram_tensor("beta", [1, D], F32, kind="ExternalInput").ap()
    ident_in = nc.dram_tensor("ident_in", [P, P], F32, kind="ExternalInput").ap()
    ones_in = nc.dram_tensor("ones_in", [1, P], F32, kind="ExternalInput").ap()

    # attention probabilities in TRANSPOSED (k-major) per-head layout;
    # host assembles [q,k] with a numpy transpose.
    attn_t = nc.dram_tensor("attn_t", [B, 2, S, S], F32, kind="ExternalOutput").ap()
    y_o = nc.dram_tensor("y_o", [TSH, D], F32, kind="ExternalOutput").ap()

    with tile.TileContext(nc) as tc:
        with tc.tile_pool(name="big", bufs=1) as big, \
             tc.tile_pool(name="wts", bufs=1) as wts, \
             tc.tile_pool(name="etp", bufs=18) as etp, \
             tc.tile_pool(name="atp", bufs=4) as atp, \
             tc.tile_pool(name="sm", bufs=8) as sm, \
             tc.tile_pool(name="ps", bufs=1, space="PSUM") as ps, \
             tc.tile_pool(name="dram", bufs=1, space="DRAM") as dram:

            # ---- persistent SBUF ----
            qt_sb = big.tile([P, T], F32R)            # Q^T  [chan, tok]
            kt_sb = big.tile([P, T], F32R)            # K^T  [chan, tok]
            # V with a ones column per head: [tok%128, tok//128, head, 1+64]
            v_sb = big.tile([P, NTT, 2, 1 + DK], F32R)
            gam_bc = big.tile([P, D], F32)
            bet_bc = big.tile([P, D], F32)

            wq_sb = wts.tile([P, DC, P], F32R)
            wk_sb = wts.tile([P, DC, P], F32R)
            wv_sb = wts.tile([P, DC, P], F32R)
            ones_sb = wts.tile([1, P], F32R)
            ident = wts.tile([P, P], F32R)
            bo_sb = wts.tile([1, D], F32R)
            gam_sb = wts.tile([1, D], F32R)
            bet_sb = wts.tile([1, D], F32R)
            eps_col = wts.tile([P, 1], F32)

            nc.sync.dma_start(wq_sb[:], wqT.rearrange("(dc p) c -> p dc c", p=P).bitcast(F32R))
            nc.sync.dma_start(wk_sb[:], wkT.rearrange("(dc p) c -> p dc c", p=P).bitcast(F32R))
            nc.sync.dma_start(wv_sb[:], wvT.rearrange("(dc p) c -> p dc c", p=P).bitcast(F32R))
            nc.sync.dma_start(bo_sb[:], bo.bitcast(F32R))
            nc.sync.dma_start(gam_sb[:], gamma.bitcast(F32R))
            nc.sync.dma_start(bet_sb[:], beta.bitcast(F32R))
            nc.sync.dma_start(ones_sb[:], ones_in.bitcast(F32R))
            nc.sync.dma_start(ident[:], ident_in.bitcast(F32R))
            nc.gpsimd.memset(eps_col[:], LN_EPS)
            nc.gpsimd.memset(v_sb[:, :, :, 0:1].bitcast(U32), ONE_U32)

            # ---- Stage A: Q^T, K^T (chan-major) and V (token-major) ----
            with tc.tile_pool(name="xtp", bufs=8) as xtp:
                for tb in range(8):           # token blocks of 512
                    xts = []
                    for dc in range(DC):
                        xt = xtp.tile([P, 512], F32R, name=f"xt_{tb}_{dc}",
                                      tag="xt")
                        nc.sync.dma_start(
                            xt[:], xT[dc * P:(dc + 1) * P,
                                      tb * 512:(tb + 1) * 512].bitcast(F32R))
                        xts.append(xt)
                    qp = ps.tile([P, 512], F32, tag="ps512a", name=f"qp{tb}",
                                 bufs=2)
                    kp = ps.tile([P, 512], F32, tag="ps512a", name=f"kp{tb}",
                                 bufs=2)
                    for dc in range(DC):
                        nc.tensor.matmul(qp[:], lhsT=wq_sb[:, dc, :],
                                         rhs=xts[dc][:],
                                         start=(dc == 0), stop=(dc == DC - 1))
                    for dc in range(DC):
                        nc.tensor.matmul(kp[:], lhsT=wk_sb[:, dc, :],
                                         rhs=xts[dc][:],
                                         start=(dc == 0), stop=(dc == DC - 1))
                    nc.vector.tensor_copy(qt_sb[:, tb * 512:(tb + 1) * 512], qp[:])
                    nc.vector.tensor_copy(kt_sb[:, tb * 512:(tb + 1) * 512], kp[:])
                    for i in range(4):        # token tiles of 128 in the block
                        vp = ps.tile([P, 512], F32, tag="ps512b",
                                     name=f"vp{tb}_{i}", bufs=2)
                        for dc in range(DC):
                            nc.tensor.matmul(vp[:, 0:P],
                                             lhsT=xts[dc][:, i * P:(i + 1) * P],
                                             rhs=wv_sb[:, dc, :],
                                             start=(dc == 0),
                                             stop=(dc == DC - 1))
                        nc.vector.tensor_copy(
                            v_sb[:, tb * 4 + i, :, 1:1 + DK],
                            vp[:, 0:P].rearrange("p (h c) -> p h c", h=2))

            # Wo^T loaded after stage A so its SBUF reuses the freed xt pool
            wop_cm = tc.tile_pool(name="wop", bufs=1)
            wop = wop_cm.__enter__()
            wo_sb = wop.tile([P, DC, D], F32R)        # Wo^T [c%128, c//128, d]
            nc.sync.dma_start(
                wo_sb[:], woT.rearrange("(dc p) d -> p dc d", p=P).bitcast(F32R))

            a2a_in = dram.tile([N_CORES, P, TSH], F32)
            a2a_out = dram.tile([N_CORES, P, TSH], F32)

            # ---- Stage B: S^T, exp, attn@V (+sums), normalize, outputs ----
            for b in range(2):
                for h in range(2):
                    ch0 = h * DK
                    for qb in range(KB):      # q blocks of 512
                        q_lo = b * S + qb * 512
                        # cxp row 0 = softmax row sums (ones column of V),
                        # rows 1..64 = ctx^T for this head.
                        cxp = ps.tile([1 + DK, 512], F32, tag="ps512b",
                                      name=f"cx{b}{h}{qb}", bufs=2)
                        ets = []
                        for kt in range(QT):  # k tiles of 128
                            st = ps.tile([P, 512], F32, tag="ps512a",
                                         name=f"st{b}{h}{qb}_{kt}", bufs=2)
                            nc.tensor.matmul(
                                st[:],
                                lhsT=kt_sb[ch0:ch0 + DK,
                                           b * S + kt * P: b * S + (kt + 1) * P],
                                rhs=qt_sb[ch0:ch0 + DK, q_lo:q_lo + 512],
                                start=True, stop=True)
                            et = etp.tile([P, 512], F32R,
                                          name=f"et{b}{h}{qb}_{kt}", tag="et")
                            nc.scalar.activation(et[:], st[:], EXP, scale=SCALE)
                            nc.tensor.matmul(
                                cxp[:],
                                lhsT=v_sb[:, b * QT + kt, h, :],
                                rhs=et[:],
                                start=(kt == 0), stop=(kt == QT - 1))
                            ets.append(et)
                        # reciprocal of row sums -> broadcast to 128 partitions
                        rrow = sm.tile([1, 512], F32, name=f"rr{b}{h}{qb}",
                                       tag="rr", bufs=2)
                        with nc.allow_low_precision(
                                reason="f32r rounding of softmax reciprocal"):
                            nc.vector.reciprocal(rrow[:].bitcast(F32R),
                                                 cxp[0:1, :])
                        bcp = ps.tile([P, 512], F32, tag="mix",
                                      name=f"bc{b}{h}{qb}", bufs=2)
                        nc.tensor.matmul(bcp[:], lhsT=ones_sb[:],
                                         rhs=rrow[:].bitcast(F32R),
                                         start=True, stop=True)
                        bc_sb = etp.tile([P, 512], F32, name=f"bcs{b}{h}{qb}",
                                         tag="bcs", bufs=2)
                        nc.scalar.copy(bc_sb[:], bcp[:])
                        # normalized attention tiles -> HBM (k-major layout)
                        for kt in range(QT):
                            at = atp.tile([P, 512], F32, name=f"at{b}{h}{qb}_{kt}",
                                          tag="at")
                            nc.vector.tensor_mul(at[:], ets[kt][:].bitcast(F32),
                                                 bc_sb[:])
                            nc.sync.dma_start(
                                attn_t[b, h, kt * P:(kt + 1) * P,
                                       qb * 512:(qb + 1) * 512], at[:])
                        # normalized ctx^T -> AllToAll staging
                        cst = etp.tile([1 + DK, 512], F32, name=f"cs{b}{h}{qb}",
                                       tag="cst", bufs=2)
                        nc.vector.tensor_mul(cst[:], cxp[:], bc_sb[0:1 + DK, :])
                        nc.sync.dma_start(
                            a2a_in[b * KB + qb, ch0:ch0 + DK, :], cst[1:1 + DK, :])

            # ---- AllToAll: channel-sharded ctx^T -> token-sharded full ctx^T
            # in[j] = my 128 channels for token block j; out[rk] = rank rk's
            # 128 channels (= global channels 128*rk..) for MY token shard.
            nc.gpsimd.collective_compute(
                "AllToAll", ALU.bypass,
                replica_groups=[list(range(N_CORES))],
                ins=[a2a_in.opt()], outs=[a2a_out.opt()])

            # gamma/beta broadcast to all partitions (PE ones-matmul)
            for name, src, dst in (("g", gam_sb, gam_bc), ("bt", bet_sb, bet_bc)):
                gbp = ps.tile([P, 1024], F32, tag="mix", name=f"gbp{name}",
                              bufs=2)
                for half in range(2):
                    nc.tensor.matmul(gbp[:, half * 512:(half + 1) * 512],
                                     lhsT=ones_sb[:],
                                     rhs=src[0:1, half * 512:(half + 1) * 512],
                                     start=True, stop=True)
                nc.vector.tensor_copy(dst[:], gbp[:])

            # ---- Stage D/E: Wo + bias + residual + LayerNorm on row shard ----
            for i in range(4):       # token tiles of 128 in the shard
                op = ps.tile([P, 1024], F32, tag="mix", name=f"op{i}", bufs=2)
                ctx_ts = []
                for rk in range(N_CORES):
                    ct = sm.tile([P, P], F32R, name=f"ct{i}_{rk}", tag="ct",
                                 bufs=16)
                    nc.sync.dma_start(
                        ct[:], a2a_out[rk, :, i * P:(i + 1) * P].bitcast(F32R))
                    ctx_ts.append(ct)
                xr_t = etp.tile([P, D], F32R, name=f"xr{i}", tag="xr", bufs=2)
                nc.sync.dma_start(xr_t[:], xrows[i * P:(i + 1) * P, :].bitcast(F32R))
                for half in range(2):
                    hs = slice(half * 512, (half + 1) * 512)
                    for rk in range(N_CORES):
                        nc.tensor.matmul(op[:, hs], lhsT=ctx_ts[rk][:],
                                         rhs=wo_sb[:, rk, hs],
                                         start=(rk == 0), stop=False)
                    nc.tensor.matmul(op[:, hs], lhsT=ones_sb[:],
                                     rhs=bo_sb[0:1, hs],
                                     start=False, stop=False)
                    nc.tensor.matmul(op[:, hs], lhsT=ident[:],
                                     rhs=xr_t[:, hs],
                                     start=False, stop=True)
                # LayerNorm over free dim (D=1024)
                musum = sm.tile([P, 1], F32, name=f"musum{i}", tag="mu")
                nc.vector.reduce_sum(musum[:], op[:], axis=mybir.AxisListType.X)
                mu = sm.tile([P, 1], F32, name=f"mu{i}", tag="mu")
                nc.vector.tensor_scalar_mul(mu[:], musum[:], 1.0 / D)
                oc = atp.tile([P, D], F32, name=f"oc{i}", tag="oc", bufs=2)
                nc.vector.tensor_scalar(oc[:], op[:], mu[:], None,
                                        op0=ALU.subtract)
                sq = atp.tile([P, D], F32, name=f"sq{i}", tag="oc", bufs=2)
                sqsum = sm.tile([P, 1], F32, name=f"sqsum{i}", tag="mu")
                nc.vector.scalar_tensor_tensor(
                    sq[:], in0=oc[:], scalar=0.0, in1=oc[:],
                    op0=ALU.add, op1=ALU.mult, accum_out=sqsum[:])
                sd = sm.tile([P, 1], F32, name=f"sd{i}", tag="mu")
                nc.scalar.activation(sd[:], sqsum[:], SQRT,
                                     scale=1.0 / D, bias=eps_col[:])
                rstd = sm.tile([P, 1], F32, name=f"rstd{i}", tag="mu")
                nc.vector.reciprocal(rstd[:], sd[:])
                y_sb = etp.tile([P, D], F32, name=f"y{i}", tag="y", bufs=2)
                nc.vector.scalar_tensor_tensor(
                    y_sb[:], in0=oc[:], scalar=rstd[:], in1=gam_bc[:],
                    op0=ALU.mult, op1=ALU.mult)
                nc.vector.tensor_add(y_sb[:], y_sb[:], bet_bc[:])
                nc.sync.dma_start(y_o[i * P:(i + 1) * P, :], y_sb[:])
            wop_cm.__exit__(None, None, None)

    nc.compile()
    return nc


_NC_CACHE = {}


def _get_nc():
    if "nc" not in _NC_CACHE:
        _NC_CACHE["nc"] = build_kernel()
    return _NC_CACHE["nc"]


def make_in_maps(x, Wq, Wk, Wv, Wo, bo, gamma, beta):
    x = np.asarray(x, dtype=np.float32)
    xf = np.ascontiguousarray(x.reshape(T, D))
    xT = np.ascontiguousarray(xf.T)
    woT = np.ascontiguousarray(np.asarray(Wo, np.float32).T)
    bo2 = np.ascontiguousarray(np.asarray(bo, np.float32).reshape(1, D))
    g2 = np.ascontiguousarray(np.asarray(gamma, np.float32).reshape(1, D))
    b2 = np.ascontiguousarray(np.asarray(beta, np.float32).reshape(1, D))
    in_maps = []
    for c in range(N_CORES):
        rs = slice(2 * c * DK, 2 * (c + 1) * DK)
        in_maps.append({
            "xT": xT,
            "xrows": np.ascontiguousarray(xf[c * TSH:(c + 1) * TSH]),
            "wqT": np.ascontiguousarray(np.asarray(Wq, np.float32)[rs].T),
            "wkT": np.ascontiguousarray(np.asarray(Wk, np.float32)[rs].T),
            "wvT": np.ascontiguousarray(np.asarray(Wv, np.float32)[rs].T),
            "woT": woT,
            "bo": bo2,
            "gamma": g2,
            "beta": b2,
            "ident_in": np.eye(P, dtype=np.float32),
            "ones_in": np.ones((1, P), dtype=np.float32),
        })
    return in_maps


def assemble_outputs(results):
    y = np.concatenate([results[c]["y_o"] for c in range(N_CORES)], axis=0)
    y = y.reshape(B, S, D)
    attn = np.stack([results[c]["attn_t"] for c in range(N_CORES)], axis=0)
    # [8, B, 2, k, q] -> [B, 16, q, k]
    attn = attn.transpose(1, 0, 2, 4, 3).reshape(B, H, S, S)
    return y, attn


def kernel(x, Wq, Wk, Wv, Wo, bo, gamma, beta):
    nc = _get_nc()
    in_maps = make_in_maps(x, Wq, Wk, Wv, Wo, bo, gamma, beta)
    res = run_bass_kernel_spmd(nc, in_maps, core_ids=list(range(N_CORES)))
    return assemble_outputs(res.results)


# revision 22
# speedup vs baseline: 1.4612x; 1.0462x over previous
"""Trainium2 Bass kernel: MultiHeadSpatioTemporalAttention (16 heads, d=1024)
+ residual + LayerNorm, returning (y, attn_probs).

Sharding: tensor-parallel over heads — core c owns heads {2c, 2c+1} for both
batch elements. Q/K/V projections are column-parallel (each core computes its
128 channels for all 4096 tokens); attention + softmax + attn@V are fully
local per head; the output projection runs after an AllToAll that turns the
channel-sharded context into a token-sharded full context, so each core does
Wo + bias + residual + LayerNorm for its own 512-token row shard.

Attention is computed once, in k-major (transposed) orientation: S^T tiles
[k,q] come straight off the PE, exp runs on ScalarE, the attn@V contraction
consumes the exp tiles directly (contraction dim on partitions, no on-chip
transposes), and a ones-column prepended to V yields the softmax row sums in
the same matmul. Normalization is one DVE multiply against a PE-broadcast
reciprocal row. The attention output is written in [k,q] layout and the host
reassembles the [q,k] tensor with a numpy transpose.

All matmuls run as float32r (TF32-like full-rate PE mode); accumulation is
fp32 in PSUM.
"""

import os
import sys

if "jax" not in sys.modules:
    # The bass SPMD runner executes through the axon PJRT proxy; make sure a
    # harness-level JAX_PLATFORMS=cpu doesn't hide the NeuronCores.
    if os.environ.get("AXON_H4_ENABLED") == "1":
        os.environ["JAX_PLATFORMS"] = "axon"

import numpy as np

import concourse.bass as bass
import concourse.mybir as mybir
import concourse.tile as tile
from concourse import bacc
from concourse.bass_utils import run_bass_kernel_spmd

F32 = mybir.dt.float32
F32R = mybir.dt.float32r
U32 = mybir.dt.uint32
EXP = mybir.ActivationFunctionType.Exp
SQRT = mybir.ActivationFunctionType.Sqrt
ALU = mybir.AluOpType

N_CORES = 8
B, S, D, H, DK = 2, 2048, 1024, 16, 64
T = B * S                # 4096 tokens total
TSH = T // N_CORES       # 512 tokens per core (output row shard)
P = 128
DC = D // P              # 8 d-chunks
QT = S // P              # 16 tiles of 128 per batch
KB = S // 512            # 4 blocks of 512 per batch
NTT = T // P             # 32 token tiles
LN_EPS = 1e-5
SCALE = 1.0 / np.sqrt(DK)  # 0.125
ONE_U32 = 0x3F800000     # 1.0f bit pattern


def build_kernel():
    nc = bacc.Bacc("TRN2", target_bir_lowering=False, debug=False,
                   enable_asserts=False, num_devices=N_CORES)

    xT = nc.dram_tensor("xT", [D, T], F32, kind="ExternalInput").ap()
    xrows = nc.dram_tensor("xrows", [TSH, D], F32, kind="ExternalInput").ap()
    wqT = nc.dram_tensor("wqT", [D, P], F32, kind="ExternalInput").ap()
    wkT = nc.dram_tensor("wkT", [D, P], F32, kind="ExternalInput").ap()
    wvT = nc.dram_tensor("wvT", [D, P], F32, kind="ExternalInput").ap()
    woT = nc.dram_tensor("woT", [D, D], F32, kind="ExternalInput").ap()
    bo = nc.dram_tensor("bo", [1, D], F32, kind="ExternalInput").ap()
    gamma = nc.dram_tensor("gamma", [1, D], F32, kind="ExternalInput").ap()
    beta = nc.dram_tensor("beta", [1, D], F32, kind="ExternalInput").ap()
    ident_in = nc.dram_tensor("ident_in", [P, P], F32, kind="ExternalInput").ap()
    ones_in = nc.dram_tensor("ones_in", [1, P], F32, kind="ExternalInput").ap()

    # attention probabilities in TRANSPOSED (k-major) per-head layout;
    # host assembles [q,k] with a numpy transpose.
    attn_t = nc.dram_tensor("attn_t", [B, 2, S, S], F32, kind="ExternalOutput").ap()
    y_o = nc.dram_tensor("y_o", [TSH, D], F32, kind="ExternalOutput").ap()

    with tile.TileContext(nc) as tc:
        with tc.tile_pool(name="big", bufs=1) as big, \
             tc.tile_pool(name="wts", bufs=1) as wts, \
             tc.tile_pool(name="etp", bufs=18) as etp, \
             tc.tile_pool(name="atp", bufs=4) as atp, \
             tc.tile_pool(name="sm", bufs=8) as sm, \
             tc.tile_pool(name="ps", bufs=1, space="PSUM") as ps, \
             tc.tile_pool(name="dram", bufs=1, space="DRAM") as dram:

            # ---- persistent SBUF ----
            qt_sb = big.tile([P, T], F32R)            # Q^T  [chan, tok]
            kt_sb = big.tile([P, T], F32R)            # K^T  [chan, tok]
            # V with a ones column per head: [tok%128, tok//128, head, 1+64]
            v_sb = big.tile([P, NTT, 2, 1 + DK], F32R)
            gam_bc = big.tile([P, D], F32)
            bet_bc = big.tile([P, D], F32)

            wq_sb = wts.tile([P, DC, P], F32R)
            wk_sb = wts.tile([P, DC, P], F32R)
            wv_sb = wts.tile([P, DC, P], F32R)
            ones_sb = wts.tile([1, P], F32R)
            ident = wts.tile([P, P], F32R)
            bo_sb = wts.tile([1, D], F32R)
            gam_sb = wts.tile([1, D], F32R)
            bet_sb = wts.tile([1, D], F32R)
            eps_col = wts.tile([P, 1], F32)

            nc.sync.dma_start(wq_sb[:], wqT.rearrange("(dc p) c -> p dc c", p=P).bitcast(F32R))
            nc.sync.dma_start(wk_sb[:], wkT.rearrange("(dc p) c -> p dc c", p=P).bitcast(F32R))
            nc.sync.dma_start(wv_sb[:], wvT.rearrange("(dc p) c -> p dc c", p=P).bitcast(F32R))
            nc.sync.dma_start(bo_sb[:], bo.bitcast(F32R))
            nc.sync.dma_start(gam_sb[:], gamma.bitcast(F32R))
            nc.sync.dma_start(bet_sb[:], beta.bitcast(F32R))
            nc.sync.dma_start(ones_sb[:], ones_in.bitcast(F32R))
            nc.sync.dma_start(ident[:], ident_in.bitcast(F32R))
            nc.gpsimd.memset(eps_col[:], LN_EPS)
            nc.gpsimd.memset(v_sb[:, :, :, 0:1].bitcast(U32), ONE_U32)

            # ---- Stage A: Q^T, K^T (chan-major) and V (token-major) ----
            with tc.tile_pool(name="xtp", bufs=8) as xtp:
                for tb in range(8):           # token blocks of 512
                    xts = []
                    for dc in range(DC):
                        xt = xtp.tile([P, 512], F32R, name=f"xt_{tb}_{dc}",
                                      tag="xt")
                        nc.sync.dma_start(
                            xt[:], xT[dc * P:(dc + 1) * P,
                                      tb * 512:(tb + 1) * 512].bitcast(F32R))
                        xts.append(xt)
                    qp = ps.tile([P, 512], F32, tag="ps512a", name=f"qp{tb}",
                                 bufs=4)
                    kp = ps.tile([P, 512], F32, tag="ps512a", name=f"kp{tb}",
                                 bufs=4)
                    for dc in range(DC):
                        nc.tensor.matmul(qp[:], lhsT=wq_sb[:, dc, :],
                                         rhs=xts[dc][:],
                                         start=(dc == 0), stop=(dc == DC - 1))
                    for dc in range(DC):
                        nc.tensor.matmul(kp[:], lhsT=wk_sb[:, dc, :],
                                         rhs=xts[dc][:],
                                         start=(dc == 0), stop=(dc == DC - 1))
                    nc.vector.tensor_copy(qt_sb[:, tb * 512:(tb + 1) * 512], qp[:])
                    nc.vector.tensor_copy(kt_sb[:, tb * 512:(tb + 1) * 512], kp[:])
                    vp = ps.tile([P, 512], F32, tag="ps512b",
                                 name=f"vp{tb}", bufs=2)
                    for dc in range(DC):
                        nc.tensor.matmul(vp[:], lhsT=wv_sb[:, dc, :],
                                         rhs=xts[dc][:],
                                         start=(dc == 0), stop=(dc == DC - 1))
                    vt = etp.tile([P, 512], F32R, name=f"vt{tb}", tag="et")
                    nc.vector.tensor_copy(vt[:], vp[:])
                    for i in range(4):        # transpose back to token-major
                        vtp = ps.tile([P, 512], F32, tag="ps512b",
                                      name=f"vtp{tb}_{i}", bufs=2)
                        nc.tensor.transpose(vtp[:, 0:P].bitcast(F32R),
                                            vt[:, i * P:(i + 1) * P], ident[:])
                        nc.vector.tensor_copy(
                            v_sb[:, tb * 4 + i, :, 1:1 + DK],
                            vtp[:, 0:P].bitcast(F32R)
                               .rearrange("p (h c) -> p h c", h=2))

            # Wo^T loaded after stage A so its SBUF reuses the freed xt pool
            wop_cm = tc.tile_pool(name="wop", bufs=1)
            wop = wop_cm.__enter__()
            wo_sb = wop.tile([P, DC, D], F32R)        # Wo^T [c%128, c//128, d]
            nc.sync.dma_start(
                wo_sb[:], woT.rearrange("(dc p) d -> p dc d", p=P).bitcast(F32R))

            a2a_in = dram.tile([N_CORES, P, TSH], F32)
            a2a_out = dram.tile([N_CORES, P, TSH], F32)

            # ---- Stage B: S^T, exp, attn@V (+sums), normalize, outputs ----
            for b in range(2):
                for h in range(2):
                    ch0 = h * DK
                    for qb in range(KB):      # q blocks of 512
                        q_lo = b * S + qb * 512
                        # cxp row 0 = softmax row sums (ones column of V),
                        # rows 1..64 = ctx^T for this head.
                        cxp = ps.tile([1 + DK, 512], F32, tag="ps512b",
                                      name=f"cx{b}{h}{qb}", bufs=2)
                        ets = []
                        for kt in range(QT):  # k tiles of 128
                            st = ps.tile([P, 512], F32, tag="ps512a",
                                         name=f"st{b}{h}{qb}_{kt}", bufs=4)
                            nc.tensor.matmul(
                                st[:],
                                lhsT=kt_sb[ch0:ch0 + DK,
                                           b * S + kt * P: b * S + (kt + 1) * P],
                                rhs=qt_sb[ch0:ch0 + DK, q_lo:q_lo + 512],
                                start=True, stop=True)
                            et = etp.tile([P, 512], F32R,
                                          name=f"et{b}{h}{qb}_{kt}", tag="et")
                            nc.scalar.activation(et[:], st[:], EXP, scale=SCALE)
                            nc.tensor.matmul(
                                cxp[:],
                                lhsT=v_sb[:, b * QT + kt, h, :],
                                rhs=et[:],
                                start=(kt == 0), stop=(kt == QT - 1))
                            ets.append(et)
                        # reciprocal of row sums -> broadcast to 128 partitions
                        rrow = sm.tile([1, 512], F32, name=f"rr{b}{h}{qb}",
                                       tag="rr", bufs=2)
                        with nc.allow_low_precision(
                                reason="f32r rounding of softmax reciprocal"):
                            nc.vector.reciprocal(rrow[:].bitcast(F32R),
                                                 cxp[0:1, :])
                        bcp = ps.tile([P, 512], F32, tag="mix",
                                      name=f"bc{b}{h}{qb}", bufs=1)
                        nc.tensor.matmul(bcp[:], lhsT=ones_sb[:],
                                         rhs=rrow[:].bitcast(F32R),
                                         start=True, stop=True)
                        bc_sb = etp.tile([P, 512], F32, name=f"bcs{b}{h}{qb}",
                                         tag="bcs", bufs=2)
                        nc.scalar.copy(bc_sb[:], bcp[:])
                        # normalized attention tiles -> HBM (k-major layout)
                        for kt in range(QT):
                            at = atp.tile([P, 512], F32, name=f"at{b}{h}{qb}_{kt}",
                                          tag="at")
                            nc.vector.tensor_mul(at[:], ets[kt][:].bitcast(F32),
                                                 bc_sb[:])
                            nc.sync.dma_start(
                                attn_t[b, h, kt * P:(kt + 1) * P,
                                       qb * 512:(qb + 1) * 512], at[:])
                        # normalized ctx^T -> AllToAll staging
                        cst = etp.tile([1 + DK, 512], F32, name=f"cs{b}{h}{qb}",
                                       tag="cst", bufs=2)
                        nc.vector.tensor_mul(cst[:], cxp[:], bc_sb[0:1 + DK, :])
                        nc.sync.dma_start(
                            a2a_in[b * KB + qb, ch0:ch0 + DK, :], cst[1:1 + DK, :])

            # ---- AllToAll: channel-sharded ctx^T -> token-sharded full ctx^T
            # in[j] = my 128 channels for token block j; out[rk] = rank rk's
            # 128 channels (= global channels 128*rk..) for MY token shard.
            nc.gpsimd.collective_compute(
                "AllToAll", ALU.bypass,
                replica_groups=[list(range(N_CORES))],
                ins=[a2a_in.opt()], outs=[a2a_out.opt()])

            # gamma/beta broadcast to all partitions (PE ones-matmul)
            for name, src, dst in (("g", gam_sb, gam_bc), ("bt", bet_sb, bet_bc)):
                gbp = ps.tile([P, 1024], F32, tag="mix", name=f"gbp{name}",
                              bufs=1)
                for half in range(2):
                    nc.tensor.matmul(gbp[:, half * 512:(half + 1) * 512],
                                     lhsT=ones_sb[:],
                                     rhs=src[0:1, half * 512:(half + 1) * 512],
                                     start=True, stop=True)
                nc.vector.tensor_copy(dst[:], gbp[:])

            # ---- Stage D/E: Wo + bias + residual + LayerNorm on row shard ----
            for i in range(4):       # token tiles of 128 in the shard
                op = ps.tile([P, 1024], F32, tag="mix", name=f"op{i}", bufs=1)
                ctx_ts = []
                for rk in range(N_CORES):
                    ct = sm.tile([P, P], F32R, name=f"ct{i}_{rk}", tag="ct",
                                 bufs=16)
                    nc.sync.dma_start(
                        ct[:], a2a_out[rk, :, i * P:(i + 1) * P].bitcast(F32R))
                    ctx_ts.append(ct)
                xr_t = etp.tile([P, D], F32R, name=f"xr{i}", tag="xr", bufs=2)
                nc.sync.dma_start(xr_t[:], xrows[i * P:(i + 1) * P, :].bitcast(F32R))
                for half in range(2):
                    hs = slice(half * 512, (half + 1) * 512)
                    for rk in range(N_CORES):
                        nc.tensor.matmul(op[:, hs], lhsT=ctx_ts[rk][:],
                                         rhs=wo_sb[:, rk, hs],
                                         start=(rk == 0), stop=False)
                    nc.tensor.matmul(op[:, hs], lhsT=ones_sb[:],
                                     rhs=bo_sb[0:1, hs],
                                     start=False, stop=False)
                    nc.tensor.matmul(op[:, hs], lhsT=ident[:],
                                     rhs=xr_t[:, hs],
                                     start=False, stop=True)
                # LayerNorm over free dim (D=1024)
                musum = sm.tile([P, 1], F32, name=f"musum{i}", tag="mu")
                nc.vector.reduce_sum(musum[:], op[:], axis=mybir.AxisListType.X)
                mu = sm.tile([P, 1], F32, name=f"mu{i}", tag="mu")
                nc.vector.tensor_scalar_mul(mu[:], musum[:], 1.0 / D)
                oc = atp.tile([P, D], F32, name=f"oc{i}", tag="oc", bufs=2)
                nc.vector.tensor_scalar(oc[:], op[:], mu[:], None,
                                        op0=ALU.subtract)
                sq = atp.tile([P, D], F32, name=f"sq{i}", tag="oc", bufs=2)
                sqsum = sm.tile([P, 1], F32, name=f"sqsum{i}", tag="mu")
                nc.vector.scalar_tensor_tensor(
                    sq[:], in0=oc[:], scalar=0.0, in1=oc[:],
                    op0=ALU.add, op1=ALU.mult, accum_out=sqsum[:])
                sd = sm.tile([P, 1], F32, name=f"sd{i}", tag="mu")
                nc.scalar.activation(sd[:], sqsum[:], SQRT,
                                     scale=1.0 / D, bias=eps_col[:])
                rstd = sm.tile([P, 1], F32, name=f"rstd{i}", tag="mu")
                nc.vector.reciprocal(rstd[:], sd[:])
                y_sb = etp.tile([P, D], F32, name=f"y{i}", tag="y", bufs=2)
                nc.vector.scalar_tensor_tensor(
                    y_sb[:], in0=oc[:], scalar=rstd[:], in1=gam_bc[:],
                    op0=ALU.mult, op1=ALU.mult)
                nc.vector.tensor_add(y_sb[:], y_sb[:], bet_bc[:])
                nc.sync.dma_start(y_o[i * P:(i + 1) * P, :], y_sb[:])
            wop_cm.__exit__(None, None, None)

    nc.compile()
    return nc


_NC_CACHE = {}


def _get_nc():
    if "nc" not in _NC_CACHE:
        _NC_CACHE["nc"] = build_kernel()
    return _NC_CACHE["nc"]


def make_in_maps(x, Wq, Wk, Wv, Wo, bo, gamma, beta):
    x = np.asarray(x, dtype=np.float32)
    xf = np.ascontiguousarray(x.reshape(T, D))
    xT = np.ascontiguousarray(xf.T)
    woT = np.ascontiguousarray(np.asarray(Wo, np.float32).T)
    bo2 = np.ascontiguousarray(np.asarray(bo, np.float32).reshape(1, D))
    g2 = np.ascontiguousarray(np.asarray(gamma, np.float32).reshape(1, D))
    b2 = np.ascontiguousarray(np.asarray(beta, np.float32).reshape(1, D))
    in_maps = []
    for c in range(N_CORES):
        rs = slice(2 * c * DK, 2 * (c + 1) * DK)
        in_maps.append({
            "xT": xT,
            "xrows": np.ascontiguousarray(xf[c * TSH:(c + 1) * TSH]),
            "wqT": np.ascontiguousarray(np.asarray(Wq, np.float32)[rs].T),
            "wkT": np.ascontiguousarray(np.asarray(Wk, np.float32)[rs].T),
            "wvT": np.ascontiguousarray(np.asarray(Wv, np.float32)[rs].T),
            "woT": woT,
            "bo": bo2,
            "gamma": g2,
            "beta": b2,
            "ident_in": np.eye(P, dtype=np.float32),
            "ones_in": np.ones((1, P), dtype=np.float32),
        })
    return in_maps


def assemble_outputs(results):
    y = np.concatenate([results[c]["y_o"] for c in range(N_CORES)], axis=0)
    y = y.reshape(B, S, D)
    attn = np.stack([results[c]["attn_t"] for c in range(N_CORES)], axis=0)
    # [8, B, 2, k, q] -> [B, 16, q, k]
    attn = attn.transpose(1, 0, 2, 4, 3).reshape(B, H, S, S)
    return y, attn


def kernel(x, Wq, Wk, Wv, Wo, bo, gamma, beta):
    nc = _get_nc()
    in_maps = make_in_maps(x, Wq, Wk, Wv, Wo, bo, gamma, beta)
    res = run_bass_kernel_spmd(nc, in_maps, core_ids=list(range(N_CORES)))
    return assemble_outputs(res.results)
